# revision 16
# baseline (speedup 1.0000x reference)
"""GCN (2-layer GCNConv + mean readout + sigmoid head) on 8 Trainium2 NeuronCores.

Strategy (graph/data parallel, dst-sharded):
  - Nodes are permuted (round-robin by in-degree) into NB = n_cores*nblk blocks of
    128 so every block has ~equal in-edge count; each core owns nblk blocks.
  - Per layer: H' = (D^-1/2 Z) @ W computed node-sharded on PE (bf16); an fp8
    copy (scaled 16x to dodge fp8 subnormals) is AllGathered in NCH node-range
    chunks so each chunk's collective fires as soon as its inputs exist and
    overlaps aggregation of earlier chunks (Shared outputs, one writer per
    table; chunk tables keep gather row ids int16).
  - Per dst-block: one dma_gather per chunk of the source rows; one-hot
    selection matrices (DVE is_equal vs an iota row, fp8) contracted on PE
    form segment sums in PSUM. Self-loops via a 16*I matmul on the local bf16
    shard; bias via a K=1 matmul of 16*(1/dinv) x bias_row; the post-op
    collapses to one ACT Relu with (dinv/16) scale.
  - Gathers round-robin over 4 SWDGE queues (desc-gen runs on one Q7 core pair
    per instruction; the GPSIMD window keeps ~2 in flight). Chunk-k gathers
    lead the block's matmuls by (NCH-1-k)*LAG blocks, so late-arriving chunk
    tables never stall the gather stream at the queue head.
  - Layer-2 transpose + dense + AllGather are issued per chunk from inside the
    layer-1 aggregation loop, hiding the whole mid phase under the gathers.
  - Readout: per-block column sums via matmul against a pad-mask vector,
    accumulated in PSUM; final cross-core reduce + fc + sigmoid on host.
"""

import math

import numpy as np
import ml_dtypes

BF16 = ml_dtypes.bfloat16

# Problem constants (hardcoded per contract; kernel.py must be self-contained).
N = 50000
E = 800000
IN_DIM = 512
HID = 256
N_CORES = 8
P = 128
NQ = 4        # SWDGE queues (gather desc-gen parallelism)
NCH = 3       # AllGather node-range chunks
LAG = 9       # per-chunk gather lead (blocks)
FP8_SCALE = 16.0


def _chunk_sizes(nblk):
    base, rem = divmod(nblk, NCH)
    sizes = [base + (1 if k < rem else 0) for k in range(NCH)]
    bnds = [0]
    for s in sizes:
        bnds.append(bnds[-1] + s)
    return sizes, bnds


def _wrap_idx(flat):
    """[L] int -> [128, L/16] int16 in the SWDGE wrapped layout."""
    L = len(flat)
    assert L % 16 == 0
    w = flat.reshape(L // 16, 16).T  # value i at [i%16, i//16]
    return np.ascontiguousarray(np.tile(w, (8, 1)).astype(np.int16))


# --------------------------------------------------------------------------- #
# Host-side preprocessing
# --------------------------------------------------------------------------- #

def _preprocess(x, edge_index, W1, b1, W2, b2):
    n, in_dim = x.shape
    hid = W1.shape[1]
    src = np.asarray(edge_index[0], dtype=np.int64)
    dst = np.asarray(edge_index[1], dtype=np.int64)

    deg_in = np.bincount(dst, minlength=n)
    deg = deg_in.astype(np.float64) + 1.0  # + self loop
    dinv = (1.0 / np.sqrt(deg)).astype(np.float32)

    nblk = math.ceil(n / (N_CORES * P))          # blocks per core
    NB = N_CORES * nblk                          # total blocks
    npad = NB * P
    nshard = nblk * P
    sizes, bnds = _chunk_sizes(nblk)
    assert N_CORES * max(sizes) * P <= 32767, "chunk table rows must fit int16"

    # Balance blocks: deal nodes round-robin across blocks in desc in-degree
    # order -> every block gets ~equal total in-degree.
    order = np.argsort(-deg_in, kind="stable")
    i = np.arange(n)
    new_id = np.empty(n, dtype=np.int64)
    new_id[order] = (i % NB) * P + (i // NB)

    # Edge arrays in permuted space, sorted by (dst block, src chunk).
    s_new = new_id[src]
    d_new = new_id[dst]
    blk_id = d_new // P
    s_rank = s_new // nshard
    s_pos = s_new % nshard
    chunk_of = np.zeros(len(s_pos), dtype=np.int64)
    s_cidx = np.zeros(len(s_pos), dtype=np.int64)
    for k in range(NCH):
        m = (s_pos >= bnds[k] * P) & (s_pos < bnds[k + 1] * P)
        chunk_of[m] = k
        s_cidx[m] = s_rank[m] * sizes[k] * P + (s_pos[m] - bnds[k] * P)
    skey = blk_id * NCH + chunk_of
    eorder = np.argsort(skey, kind="stable")
    s_cidx = s_cidx[eorder]
    d_new = d_new[eorder]
    key_sorted = skey[eorder]

    cnt = np.bincount(key_sorted, minlength=NCH * NB).reshape(NB, NCH)
    cs = [max(1, int(math.ceil(cnt[:, k].max() / P))) for k in range(NCH)]
    c_tot = sum(cs)
    c_off = [0]
    for c in cs:
        c_off.append(c_off[-1] + c)

    # Per-(block, chunk) padded slots.
    idxs = [np.zeros((NB, cs[k] * P), dtype=np.int64) for k in range(NCH)]
    dst_arr = np.full((NB, c_tot * P), -1.0, dtype=np.float32)

    starts = np.zeros(NCH * NB + 1, dtype=np.int64)
    np.cumsum(cnt.reshape(-1), out=starts[1:])
    pos = np.arange(len(s_cidx)) - starts[key_sorted]
    for k in range(NCH):
        m = key_sorted % NCH == k
        bk = key_sorted[m] // NCH
        idxs[k][bk, pos[m]] = s_cidx[m]
        dst_arr[bk, c_off[k] * P + pos[m]] = (d_new[m] % P).astype(np.float32)

    dst_arr = dst_arr.reshape(NB, c_tot, P)

    # x' = dinv * x, permuted, padded, per-core transposed, bf16.
    xp = np.zeros((npad, in_dim), dtype=np.float32)
    xp[new_id] = x * dinv[:, None]

    dinv_pad = np.zeros(npad, dtype=np.float32)
    dinv_pad[new_id] = dinv
    binv_pad = np.zeros(npad, dtype=np.float32)
    binv_pad[new_id] = FP8_SCALE / dinv
    mask_pad = np.zeros(npad, dtype=np.float32)
    mask_pad[new_id] = 1.0

    iota = np.broadcast_to(np.arange(P, dtype=np.float32), (P, P))
    ident = np.eye(P, dtype=np.float32) * FP8_SCALE

    common = {
        "w1": np.ascontiguousarray(W1.astype(BF16)),
        "w2": np.ascontiguousarray(W2.astype(BF16)),
        "brow1": np.ascontiguousarray(b1.reshape(1, hid).astype(BF16)),
        "brow2": np.ascontiguousarray(b2.reshape(1, hid).astype(BF16)),
        "iota": np.ascontiguousarray(iota.astype(BF16)),
        "ident16": np.ascontiguousarray(ident.astype(BF16)),
    }

    in_maps = []
    for c in range(N_CORES):
        lo_b, hi_b = c * nblk, (c + 1) * nblk
        lo_n, hi_n = c * nshard, (c + 1) * nshard
        m = dict(common)
        m["xT"] = np.ascontiguousarray(xp[lo_n:hi_n].T.astype(BF16))
        for k in range(NCH):
            m[f"idx{k}"] = _wrap_idx(idxs[k][lo_b:hi_b].reshape(-1))
        # [nblk, c_tot, P] -> [P, nblk*c_tot]
        m["dstf"] = np.ascontiguousarray(
            dst_arr[lo_b:hi_b].transpose(2, 0, 1).reshape(P, nblk * c_tot).astype(BF16))
        dvc = dinv_pad[lo_n:hi_n].reshape(nblk, P).T
        m["dinv"] = np.ascontiguousarray(dvc.astype(np.float32))
        m["dinvs"] = np.ascontiguousarray((dvc / FP8_SCALE).astype(np.float32))
        m["dinvx"] = np.ascontiguousarray((dvc * FP8_SCALE).astype(np.float32))
        m["binv"] = np.ascontiguousarray(
            binv_pad[lo_n:hi_n].reshape(1, nshard).astype(BF16))
        m["maskc"] = np.ascontiguousarray(
            mask_pad[lo_n:hi_n].reshape(nblk, P).T.astype(BF16))
        in_maps.append(m)

    meta = dict(nblk=nblk, cs=tuple(cs), in_dim=in_dim, hid=hid, n=n)
    return in_maps, meta


# --------------------------------------------------------------------------- #
# Device program
# --------------------------------------------------------------------------- #

def _build_nc(nblk, cs, in_dim, hid):
    from contextlib import ExitStack

    from concourse import bass, mybir, bacc
    import concourse.tile as tile

    dt = mybir.dt
    nshard = nblk * P
    sizes, bnds = _chunk_sizes(nblk)
    KIN = in_dim // P
    KH = hid // P
    cs = list(cs)
    c_tot = sum(cs)
    c_off = [0]
    for c in cs:
        c_off.append(c_off[-1] + c)
    TLAG = (NCH - 1) * LAG

    nc = bacc.Bacc(None, target_bir_lowering=False, num_devices=N_CORES,
                   num_swdge_queues=NQ)

    xT = nc.dram_tensor("xT", [in_dim, nshard], dt.bfloat16, kind="ExternalInput")
    w1 = nc.dram_tensor("w1", [in_dim, hid], dt.bfloat16, kind="ExternalInput")
    w2 = nc.dram_tensor("w2", [hid, hid], dt.bfloat16, kind="ExternalInput")
    brow1 = nc.dram_tensor("brow1", [1, hid], dt.bfloat16, kind="ExternalInput")
    brow2 = nc.dram_tensor("brow2", [1, hid], dt.bfloat16, kind="ExternalInput")
    iota = nc.dram_tensor("iota", [P, P], dt.bfloat16, kind="ExternalInput")
    ident16 = nc.dram_tensor("ident16", [P, P], dt.bfloat16, kind="ExternalInput")
    idx_t = [nc.dram_tensor(f"idx{k}", [P, nblk * cs[k] * 8], dt.int16,
                            kind="ExternalInput") for k in range(NCH)]
    dstf = nc.dram_tensor("dstf", [P, nblk * c_tot], dt.bfloat16, kind="ExternalInput")
    dinv = nc.dram_tensor("dinv", [P, nblk], dt.float32, kind="ExternalInput")
    dinvs = nc.dram_tensor("dinvs", [P, nblk], dt.float32, kind="ExternalInput")
    dinvx = nc.dram_tensor("dinvx", [P, nblk], dt.float32, kind="ExternalInput")
    binv = nc.dram_tensor("binv", [1, nshard], dt.bfloat16, kind="ExternalInput")
    maskc = nc.dram_tensor("maskc", [P, nblk], dt.bfloat16, kind="ExternalInput")
    out = nc.dram_tensor("partial", [P, KH], dt.float32, kind="ExternalOutput")

    with tile.TileContext(nc) as tc, ExitStack() as ctx:
        const = ctx.enter_context(tc.tile_pool(name="const", bufs=1))
        persist = ctx.enter_context(tc.tile_pool(name="persist", bufs=1))
        lhsp = ctx.enter_context(tc.tile_pool(name="lhsp", bufs=8))
        # per-chunk message pools: chunk k's gathers lead consumption by
        # (NCH-1-k)*LAG blocks, so earlier chunks need deeper rotation.
        msgps = [
            ctx.enter_context(tc.tile_pool(name=f"msgp{k}",
                                           bufs=(NCH - 1 - k) * LAG + 2))
            for k in range(NCH)
        ]
        h8p = ctx.enter_context(tc.tile_pool(name="h8p", bufs=2))
        stp = ctx.enter_context(tc.tile_pool(name="stp", bufs=3))
        zp = ctx.enter_context(tc.tile_pool(name="zp", bufs=3))
        ps_mm = ctx.enter_context(tc.tile_pool(name="ps_mm", bufs=2, space="PSUM"))
        ps_agg = ctx.enter_context(tc.tile_pool(name="ps_agg", bufs=2, space="PSUM"))
        ps_cs = ctx.enter_context(tc.tile_pool(name="ps_cs", bufs=1, space="PSUM"))
        dram = ctx.enter_context(tc.tile_pool(name="dram", bufs=1, space="DRAM"))

        # ---- persistent / constant tiles ----
        w1_sb = const.tile([P, KIN * hid], dt.bfloat16, tag="w1_sb")
        w2_sb = const.tile([P, KH * hid], dt.bfloat16, tag="w2_sb")
        brow1_sb = const.tile([1, hid], dt.bfloat16, tag="brow1_sb")
        brow2_sb = const.tile([1, hid], dt.bfloat16, tag="brow2_sb")
        iota_sb = const.tile([P, P], dt.bfloat16, tag="iota_sb")
        ident16_sb = const.tile([P, P], dt.bfloat16, tag="ident16_sb")
        idx_sb = [const.tile([P, nblk * cs[k] * 8], dt.int16, tag=f"idx{k}_sb",
                             name=f"idx{k}_sb") for k in range(NCH)]
        dst_sb = const.tile([P, nblk * c_tot], dt.bfloat16, tag="dst_sb")
        dinv_sb = const.tile([P, nblk], dt.float32, tag="dinv_sb")
        dinvs_sb = const.tile([P, nblk], dt.float32, tag="dinvs_sb")
        dinvx_sb = const.tile([P, nblk], dt.float32, tag="dinvx_sb")
        binv_sb = const.tile([1, nshard], dt.bfloat16, tag="binv_sb")
        mask_sb = const.tile([P, nblk], dt.bfloat16, tag="mask_sb")

        zT_sb = [persist.tile([P, KH * sizes[k] * P], dt.bfloat16,
                              tag=f"zT{k}_sb", name=f"zT{k}_sb")
                 for k in range(NCH)]
        h1_sb = persist.tile([P, nblk * hid], dt.bfloat16, tag="h1_sb")
        h2_sb = persist.tile([P, nblk * hid], dt.bfloat16, tag="h2_sb")

        hshard_d = [dram.tile([sizes[k] * P, hid], dt.float8e4,
                              tag=f"hshard{k}_d", name=f"hshard{k}_d")
                    for k in range(NCH)]
        # Shared AG outputs must each have a single writer.
        tab_d = [[dram.tile([N_CORES * sizes[k] * P, hid], dt.float8e4,
                            tag=f"t{l}{k}_d", name=f"t{l}{k}_d",
                            addr_space="Shared")
                  for k in range(NCH)] for l in range(2)]
        z1_d = [dram.tile([sizes[k] * P, hid], dt.bfloat16,
                          tag=f"z1{k}_d", name=f"z1{k}_d") for k in range(NCH)]

        # ---- constant loads ----
        nc.sync.dma_start(
            out=w1_sb[:, :].rearrange("p (k f) -> p k f", k=KIN),
            in_=w1[:, :].rearrange("(k p) f -> p k f", p=P))
        nc.sync.dma_start(
            out=w2_sb[:, :].rearrange("p (k f) -> p k f", k=KH),
            in_=w2[:, :].rearrange("(k p) f -> p k f", p=P))
        nc.sync.dma_start(out=brow1_sb[:, :], in_=brow1[:, :])
        nc.sync.dma_start(out=brow2_sb[:, :], in_=brow2[:, :])
        nc.sync.dma_start(out=iota_sb[:, :], in_=iota[:, :])
        nc.sync.dma_start(out=ident16_sb[:, :], in_=ident16[:, :])
        for k in range(NCH):
            nc.sync.dma_start(out=idx_sb[k][:, :], in_=idx_t[k][:, :])
        nc.sync.dma_start(out=dst_sb[:, :], in_=dstf[:, :])
        nc.sync.dma_start(out=dinv_sb[:, :], in_=dinv[:, :])
        nc.sync.dma_start(out=dinvs_sb[:, :], in_=dinvs[:, :])
        nc.sync.dma_start(out=dinvx_sb[:, :], in_=dinvx[:, :])
        nc.sync.dma_start(out=binv_sb[:, :], in_=binv[:, :])
        nc.sync.dma_start(out=mask_sb[:, :], in_=maskc[:, :])

        qcnt = [0]

        def next_q():
            q = qcnt[0] % NQ
            qcnt[0] += 1
            return q

        def dense_block(nb, lhs_src, w_sb, kc, scale_from_dinv, h_sb, h8c, lo_b):
            """h_sb/h8c[:, ...] = scale * (Z @ W) (bf16 / fp8 scaled 16x)."""
            ps = ps_mm.tile([P, hid], dt.float32, tag="mm")
            for k in range(kc):
                nc.tensor.matmul(
                    out=ps[:, :],
                    lhsT=lhs_src(nb, k),
                    rhs=w_sb[:, k * hid:(k + 1) * hid],
                    start=(k == 0), stop=(k == kc - 1))
            scale = dinv_sb[:, nb:nb + 1] if scale_from_dinv else 1.0
            scale16 = dinvx_sb[:, nb:nb + 1] if scale_from_dinv else FP8_SCALE
            nc.scalar.activation(
                h_sb[:, nb * hid:(nb + 1) * hid], ps[:, :],
                mybir.ActivationFunctionType.Copy, scale=scale)
            nc.scalar.activation(
                h8c[:, (nb - lo_b) * hid:(nb - lo_b + 1) * hid], ps[:, :],
                mybir.ActivationFunctionType.Copy, scale=scale16)

        def distribute(h8c, k, table_d):
            nbc = sizes[k]
            nc.sync.dma_start(
                out=hshard_d[k][:, :].rearrange("(nb p) f -> p nb f", p=P),
                in_=h8c[:, :nbc * hid].rearrange("p (nb f) -> p nb f", nb=nbc))
            nc.gpsimd.collective_compute(
                "AllGather", mybir.AluOpType.bypass,
                replica_groups=[list(range(N_CORES))],
                ins=[hshard_d[k][:, :].opt()],
                outs=[table_d[:, :].opt()])

        def aggregate(tabs, brow_sb, h_sb, z_consumer, post_block=None,
                      pre_hook=None):
            """Software-pipelined: chunk k's gather leads the matmuls by
            (NCH-1-k)*LAG blocks."""
            mts = [dict() for _ in range(NCH)]
            for i in range(nblk + TLAG):
                if pre_hook is not None:
                    pre_hook(i)
                for k in range(NCH):
                    nb = i - k * LAG
                    if not (0 <= nb < nblk):
                        continue
                    mt = msgps[k].tile([P, cs[k] * hid], dt.float8e4,
                                       tag=f"m{k}", name=f"m{k}")
                    nc.gpsimd.dma_gather(
                        out_ap=mt[:, :].rearrange("p (c f) -> p c f", c=cs[k]),
                        in_ap=tabs[k][:, :],
                        idxs_ap=idx_sb[k][:, nb * cs[k] * 8:(nb + 1) * cs[k] * 8],
                        num_idxs=cs[k] * P,
                        num_idxs_reg=cs[k] * P,
                        elem_size=hid, single_packet=False,
                        queue_num=next_q())
                    mts[k][nb] = mt
                nb = i - TLAG
                if nb < 0:
                    continue
                agg = ps_agg.tile([P, hid], dt.float32, tag="agg")
                st = stp.tile([P, c_tot * P], dt.float8e4, tag="st")
                nc.vector.tensor_tensor(
                    out=st[:, :].rearrange("p (c q) -> p c q", c=c_tot),
                    in0=dst_sb[:, nb * c_tot:(nb + 1) * c_tot]
                        .unsqueeze(2).to_broadcast([P, c_tot, P]),
                    in1=iota_sb[:, :].unsqueeze(1).to_broadcast([P, c_tot, P]),
                    op=mybir.AluOpType.is_equal)
                first = True
                for k in range(NCH):
                    mt = mts[k].pop(nb)
                    for c in range(cs[k]):
                        nc.tensor.matmul(
                            out=agg[:, :],
                            lhsT=st[:, (c_off[k] + c) * P:(c_off[k] + c + 1) * P],
                            rhs=mt[:, c * hid:(c + 1) * hid],
                            start=first, stop=False)
                        first = False
                # bias as K=1 rank-1 update: 16*(1/dinv)_col x bias_row, so the
                # final dinv/16 scale reconstitutes a plain bias add.
                nc.tensor.matmul(
                    out=agg[:, :],
                    lhsT=binv_sb[0:1, nb * P:(nb + 1) * P],
                    rhs=brow_sb[0:1, :],
                    start=False, stop=False)
                nc.tensor.matmul(
                    out=agg[:, :], lhsT=ident16_sb[:, :],
                    rhs=h_sb[:, nb * hid:(nb + 1) * hid],
                    start=False, stop=True)
                z = zp.tile([P, hid], dt.bfloat16, tag="z")
                nc.scalar.activation(
                    z[:, :], agg[:, :], mybir.ActivationFunctionType.Relu,
                    scale=dinvs_sb[:, nb:nb + 1])
                z_consumer(nb, z)
                if post_block is not None:
                    post_block(nb)

        # ================= layer 1 dense + chunked AG =================
        XB = 4  # xT blocks per lhs DMA
        xT_tiles = {}

        def xT_lhs(nb, k):
            nbg = nb - nb % XB
            key = (nbg, k)
            if key not in xT_tiles:
                w = min(XB, nblk - nbg)
                t = lhsp.tile([P, XB * P], dt.bfloat16, tag="xTt")
                nc.sync.dma_start(
                    out=t[:, :w * P],
                    in_=xT[k * P:(k + 1) * P, nbg * P:(nbg + w) * P])
                xT_tiles[key] = t
            return xT_tiles[key][:, (nb % XB) * P:(nb % XB + 1) * P]

        for k in range(NCH):
            h8c = h8p.tile([P, max(sizes) * hid], dt.float8e4, tag="h8c")
            for nb in range(bnds[k], bnds[k + 1]):
                dense_block(nb, xT_lhs, w1_sb, KIN, False, h1_sb, h8c, bnds[k])
            distribute(h8c, k, tab_d[0][k])

        # ============ layer 1 aggregation (with overlapped mid phase) ========
        def z1_write(nb, z):
            for k in range(NCH):
                if bnds[k] <= nb < bnds[k + 1]:
                    nb2 = nb - bnds[k]
                    nc.sync.dma_start(
                        out=z1_d[k][nb2 * P:(nb2 + 1) * P, :], in_=z[:, :])

        def zT_lhs(nb, k):
            for c in range(NCH):
                if bnds[c] <= nb < bnds[c + 1]:
                    nb2 = nb - bnds[c]
                    nsh = sizes[c] * P
                    return zT_sb[c][:, k * nsh + nb2 * P:k * nsh + (nb2 + 1) * P]

        h8c_pend = {}
        # trigger points inside the L1 aggregation (block index space):
        tr_T = {bnds[k + 1] - 1: k for k in range(NCH)}        # transpose
        tr_D = {}                                              # layer-2 dense
        tr_G = {}                                              # layer-2 AG
        for k in range(NCH):
            d = bnds[k + 1] + 3
            g = bnds[k + 1] + 10
            if d < nblk - 1 and k < NCH - 1:
                tr_D[d] = k
                tr_G[min(g, nblk - 1)] = k
            else:
                tr_D.setdefault(nblk - 1, k)  # last chunk: dense at the end
                # last chunk's AG is issued from the layer-2 pre_hook.

        def mid_phase(nb):
            # as each z1 chunk completes: transpose it, then (a few blocks
            # later, so the queue heads never stall) run the layer-2 dense for
            # those blocks; fire the chunk's AG once its inputs surely exist.
            if nb in tr_T:
                k = tr_T[nb]
                nsh = sizes[k] * P
                for h in range(KH):
                    nc.sync.dma_start(
                        out=zT_sb[k][:, h * nsh:(h + 1) * nsh],
                        in_=z1_d[k][:, h * P:(h + 1) * P],
                        transpose=True)
            if nb in tr_D:
                k0 = tr_D[nb]
                for k in range(k0, NCH) if nb == nblk - 1 else [k0]:
                    h8c = h8p.tile([P, max(sizes) * hid], dt.float8e4, tag="h8c")
                    for nb2 in range(bnds[k], bnds[k + 1]):
                        dense_block(nb2, zT_lhs, w2_sb, KH, True, h2_sb, h8c,
                                    bnds[k])
                    h8c_pend[k] = h8c
            if nb in tr_G:
                k = tr_G[nb]
                distribute(h8c_pend.pop(k), k, tab_d[1][k])

        aggregate(tab_d[0], brow1_sb, h1_sb, z1_write, post_block=mid_phase)

        # ================= layer 2 aggregation + readout =================
        cs_ps = [ps_cs.tile([P, 1], dt.float32, tag=f"cs{h}", name=f"cs{h}")
                 for h in range(KH)]

        def colsum(nb, z):
            for h in range(KH):
                nc.tensor.matmul(
                    out=cs_ps[h][:, :], lhsT=z[:, h * P:(h + 1) * P],
                    rhs=mask_sb[:, nb:nb + 1],
                    start=(nb == 0), stop=(nb == nblk - 1))

        def l2_pre(i):
            # remaining chunk AGs: chunk k's first gather is at i == k*LAG, so
            # issue its AG one slot earlier.
            for k in list(sorted(h8c_pend)):
                if i == max(0, k * LAG - 1):
                    distribute(h8c_pend.pop(k), k, tab_d[1][k])

        aggregate(tab_d[1], brow2_sb, h2_sb, colsum, pre_hook=l2_pre)

        out_sb = zp.tile([P, KH], dt.float32, tag="out_sb")
        for h in range(KH):
            nc.vector.tensor_copy(out=out_sb[:, h:h + 1], in_=cs_ps[h][:, :])
        nc.sync.dma_start(out=out[:, :], in_=out_sb[:, :])

    nc.compile()
    return nc


# --------------------------------------------------------------------------- #
# Entry point
# --------------------------------------------------------------------------- #

_CACHE = {}


def _run(x, edge_index, W1, b1, W2, b2, trace=False):
    from concourse.bass_utils import run_bass_kernel_spmd

    in_maps, meta = _preprocess(x, edge_index, W1, b1, W2, b2)
    key = (meta["nblk"], meta["cs"], meta["in_dim"], meta["hid"])
    if key not in _CACHE:
        _CACHE[key] = _build_nc(*key)
    nc = _CACHE[key]
    res = run_bass_kernel_spmd(
        nc, in_maps, core_ids=list(range(N_CORES)), trace=trace)
    parts = [r["partial"] for r in res.results]  # each [P, KH] f32
    colsum = np.sum(np.stack(parts), axis=0)     # [P, KH]
    g = colsum.T.reshape(-1)                     # [hid], g[h*P+p] = colsum[p, h]
    return g / float(meta["n"]), res


def kernel(x, edge_index, W1, b1, W2, b2, Wfc, bfc):
    x = np.asarray(x, dtype=np.float32)
    g, _ = _run(x, edge_index, np.asarray(W1, np.float32), np.asarray(b1, np.float32),
                np.asarray(W2, np.float32), np.asarray(b2, np.float32))
    logits = g.astype(np.float32) @ np.asarray(Wfc, np.float32) + np.asarray(bfc, np.float32)
    return (1.0 / (1.0 + np.exp(-logits))).astype(np.float32)


# revision 19
# speedup vs baseline: 1.3541x; 1.3541x over previous
"""GCN (2-layer GCNConv + mean readout + sigmoid head) on 8 Trainium2 NeuronCores.

Strategy (graph/data parallel, dst-sharded):
  - Nodes are permuted (round-robin by in-degree) into NB = n_cores*nblk blocks of
    128 so every block has ~equal in-edge count; each core owns nblk blocks.
  - Per layer: H' = (D^-1/2 Z) @ W computed node-sharded on PE (bf16); an fp8
    copy (scaled 16x to dodge fp8 subnormals) is AllGathered into a Shared
    per-layer table.
  - Per dst-block: dma_gather of the source rows (table split in two halves so
    row ids fit int16), one-hot selection matrices (DVE is_equal vs an iota
    row, fp8) contracted on PE to form segment sums in PSUM. Self-loops via a
    16*I matmul on the local bf16 shard; bias via a K=1 matmul of 16*(1/dinv)
    x bias_row; the post-op collapses to one ACT Relu with (dinv/16) scale.
  - Gathers round-robin over 4 SWDGE queues (desc-gen runs on one Q7 core pair
    per instruction; the GPSIMD window keeps ~2 in flight) -- desc generation
    is the kernel's bottleneck, everything else hides underneath it.
  - The layer-1 -> layer-2 chain (z transpose, dense-2) is partially hoisted:
    the first half's xbar transpose and dense matmuls are issued mid-way
    through the layer-1 aggregation (separate DRAM/SBUF tiles per half avoid
    false WAR serialization); only the second half's chain plus one AllGather
    remains between the layers.
  - Readout: per-block column sums via matmul against a pad-mask vector,
    accumulated in PSUM; final cross-core reduce + fc + sigmoid on host.
"""

import math

import numpy as np
import ml_dtypes

BF16 = ml_dtypes.bfloat16

# Problem constants (hardcoded per contract; kernel.py must be self-contained).
N = 50000
E = 800000
IN_DIM = 512
HID = 256
N_CORES = 8
P = 128
NQ = 4         # SWDGE queues (gather desc-gen parallelism)
MSG_BUFS = 10  # gather destination buffering depth
FP8_SCALE = 16.0


def _wrap_idx(flat):
    """[L] int -> [128, L/16] int16 in the SWDGE wrapped layout."""
    L = len(flat)
    assert L % 16 == 0
    w = flat.reshape(L // 16, 16).T  # value i at [i%16, i//16]
    return np.ascontiguousarray(np.tile(w, (8, 1)).astype(np.int16))


# --------------------------------------------------------------------------- #
# Host-side preprocessing
# --------------------------------------------------------------------------- #

def _preprocess(x, edge_index, W1, b1, W2, b2):
    n, in_dim = x.shape
    hid = W1.shape[1]
    src = np.asarray(edge_index[0], dtype=np.int64)
    dst = np.asarray(edge_index[1], dtype=np.int64)

    deg_in = np.bincount(dst, minlength=n)
    deg = deg_in.astype(np.float64) + 1.0  # + self loop
    dinv = (1.0 / np.sqrt(deg)).astype(np.float32)

    nblk = math.ceil(n / (N_CORES * P))          # blocks per core
    NB = N_CORES * nblk                          # total blocks
    npad = NB * P
    nshard = nblk * P
    half = npad // 2
    assert half <= 32767, "table half must fit int16"

    # Balance blocks: deal nodes round-robin across blocks in desc in-degree
    # order -> every block gets ~equal total in-degree.
    order = np.argsort(-deg_in, kind="stable")
    i = np.arange(n)
    new_id = np.empty(n, dtype=np.int64)
    new_id[order] = (i % NB) * P + (i // NB)

    # Edge arrays in permuted space, sorted by (dst block, src half).
    s_new = new_id[src]
    d_new = new_id[dst]
    blk_id = d_new // P
    is_hi = (s_new >= half).astype(np.int64)
    skey = blk_id * 2 + is_hi
    eorder = np.argsort(skey, kind="stable")
    s_new = s_new[eorder]
    d_new = d_new[eorder]
    key_sorted = skey[eorder]

    cnt = np.bincount(key_sorted, minlength=2 * NB).reshape(NB, 2)
    c_lo = max(1, int(math.ceil(cnt[:, 0].max() / P)))
    c_hi = max(1, int(math.ceil(cnt[:, 1].max() / P)))
    c_tot = c_lo + c_hi

    # Per-(block, half) padded slots.
    idx_lo = np.zeros((NB, c_lo * P), dtype=np.int64)
    idx_hi = np.zeros((NB, c_hi * P), dtype=np.int64)
    dst_arr = np.full((NB, c_tot * P), -1.0, dtype=np.float32)

    starts = np.zeros(2 * NB + 1, dtype=np.int64)
    np.cumsum(cnt.reshape(-1), out=starts[1:])
    pos = np.arange(len(s_new)) - starts[key_sorted]
    lo_m = key_sorted % 2 == 0
    hi_m = ~lo_m
    b_lo, b_hi = key_sorted[lo_m] // 2, key_sorted[hi_m] // 2
    idx_lo[b_lo, pos[lo_m]] = s_new[lo_m]
    idx_hi[b_hi, pos[hi_m]] = s_new[hi_m] - half
    dst_arr[b_lo, pos[lo_m]] = (d_new[lo_m] % P).astype(np.float32)
    dst_arr[b_hi, c_lo * P + pos[hi_m]] = (d_new[hi_m] % P).astype(np.float32)

    dst_arr = dst_arr.reshape(NB, c_tot, P)

    # x' = dinv * x, permuted, padded, per-core transposed, bf16.
    xp = np.zeros((npad, in_dim), dtype=np.float32)
    xp[new_id] = x * dinv[:, None]

    dinv_pad = np.zeros(npad, dtype=np.float32)
    dinv_pad[new_id] = dinv
    binv_pad = np.zeros(npad, dtype=np.float32)
    binv_pad[new_id] = FP8_SCALE / dinv
    mask_pad = np.zeros(npad, dtype=np.float32)
    mask_pad[new_id] = 1.0

    iota = np.broadcast_to(np.arange(P, dtype=np.float32), (P, P))
    ident = np.eye(P, dtype=np.float32) * FP8_SCALE

    common = {
        "w1": np.ascontiguousarray(W1.astype(BF16)),
        "w2": np.ascontiguousarray(W2.astype(BF16)),
        "brow1": np.ascontiguousarray(b1.reshape(1, hid).astype(BF16)),
        "brow2": np.ascontiguousarray(b2.reshape(1, hid).astype(BF16)),
        "iota": np.ascontiguousarray(iota.astype(BF16)),
        "ident16": np.ascontiguousarray(ident.astype(BF16)),
    }

    in_maps = []
    for c in range(N_CORES):
        lo_b, hi_b = c * nblk, (c + 1) * nblk
        lo_n, hi_n = c * nshard, (c + 1) * nshard
        m = dict(common)
        m["xT"] = np.ascontiguousarray(xp[lo_n:hi_n].T.astype(BF16))
        m["idxlo"] = _wrap_idx(idx_lo[lo_b:hi_b].reshape(-1))
        m["idxhi"] = _wrap_idx(idx_hi[lo_b:hi_b].reshape(-1))
        # [nblk, c_tot, P] -> [P, nblk*c_tot]
        m["dstf"] = np.ascontiguousarray(
            dst_arr[lo_b:hi_b].transpose(2, 0, 1).reshape(P, nblk * c_tot).astype(BF16))
        dvc = dinv_pad[lo_n:hi_n].reshape(nblk, P).T
        m["dinv"] = np.ascontiguousarray(dvc.astype(np.float32))
        m["dinvs"] = np.ascontiguousarray((dvc / FP8_SCALE).astype(np.float32))
        m["dinvx"] = np.ascontiguousarray((dvc * FP8_SCALE).astype(np.float32))
        m["binv"] = np.ascontiguousarray(
            binv_pad[lo_n:hi_n].reshape(1, nshard).astype(BF16))
        m["maskc"] = np.ascontiguousarray(
            mask_pad[lo_n:hi_n].reshape(nblk, P).T.astype(BF16))
        in_maps.append(m)

    meta = dict(nblk=nblk, c_lo=c_lo, c_hi=c_hi, in_dim=in_dim, hid=hid, n=n)
    return in_maps, meta


# --------------------------------------------------------------------------- #
# Device program
# --------------------------------------------------------------------------- #

def _build_nc(nblk, c_lo, c_hi, in_dim, hid):
    from contextlib import ExitStack

    from concourse import bass, mybir, bacc
    import concourse.tile as tile

    dt = mybir.dt
    nshard = nblk * P
    npad = N_CORES * nshard
    half = npad // 2
    KIN = in_dim // P
    KH = hid // P
    c_tot = c_lo + c_hi
    ca_blk = nblk // 2 + 2          # first-half blocks (hoisted mid chain)
    cb_blk = nblk - ca_blk
    na_sh, nb_sh = ca_blk * P, cb_blk * P

    nc = bacc.Bacc(None, target_bir_lowering=False, num_devices=N_CORES,
                   num_swdge_queues=NQ)

    xT = nc.dram_tensor("xT", [in_dim, nshard], dt.bfloat16, kind="ExternalInput")
    w1 = nc.dram_tensor("w1", [in_dim, hid], dt.bfloat16, kind="ExternalInput")
    w2 = nc.dram_tensor("w2", [hid, hid], dt.bfloat16, kind="ExternalInput")
    brow1 = nc.dram_tensor("brow1", [1, hid], dt.bfloat16, kind="ExternalInput")
    brow2 = nc.dram_tensor("brow2", [1, hid], dt.bfloat16, kind="ExternalInput")
    iota = nc.dram_tensor("iota", [P, P], dt.bfloat16, kind="ExternalInput")
    ident16 = nc.dram_tensor("ident16", [P, P], dt.bfloat16, kind="ExternalInput")
    idxlo = nc.dram_tensor("idxlo", [P, nblk * c_lo * 8], dt.int16, kind="ExternalInput")
    idxhi = nc.dram_tensor("idxhi", [P, nblk * c_hi * 8], dt.int16, kind="ExternalInput")
    dstf = nc.dram_tensor("dstf", [P, nblk * c_tot], dt.bfloat16, kind="ExternalInput")
    dinv = nc.dram_tensor("dinv", [P, nblk], dt.float32, kind="ExternalInput")
    dinvs = nc.dram_tensor("dinvs", [P, nblk], dt.float32, kind="ExternalInput")
    dinvx = nc.dram_tensor("dinvx", [P, nblk], dt.float32, kind="ExternalInput")
    binv = nc.dram_tensor("binv", [1, nshard], dt.bfloat16, kind="ExternalInput")
    maskc = nc.dram_tensor("maskc", [P, nblk], dt.bfloat16, kind="ExternalInput")
    out = nc.dram_tensor("partial", [P, KH], dt.float32, kind="ExternalOutput")

    with tile.TileContext(nc) as tc, ExitStack() as ctx:
        const = ctx.enter_context(tc.tile_pool(name="const", bufs=1))
        persist = ctx.enter_context(tc.tile_pool(name="persist", bufs=1))
        lhsp = ctx.enter_context(tc.tile_pool(name="lhsp", bufs=8))
        msgp = ctx.enter_context(tc.tile_pool(name="msgp", bufs=MSG_BUFS))
        h8p = ctx.enter_context(tc.tile_pool(name="h8p", bufs=1))
        stp = ctx.enter_context(tc.tile_pool(name="stp", bufs=3))
        zp = ctx.enter_context(tc.tile_pool(name="zp", bufs=3))
        ps_mm = ctx.enter_context(tc.tile_pool(name="ps_mm", bufs=2, space="PSUM"))
        ps_agg = ctx.enter_context(tc.tile_pool(name="ps_agg", bufs=2, space="PSUM"))
        ps_cs = ctx.enter_context(tc.tile_pool(name="ps_cs", bufs=1, space="PSUM"))
        dram = ctx.enter_context(tc.tile_pool(name="dram", bufs=1, space="DRAM"))

        # ---- persistent / constant tiles ----
        w1_sb = const.tile([P, KIN * hid], dt.bfloat16, tag="w1_sb")
        w2_sb = const.tile([P, KH * hid], dt.bfloat16, tag="w2_sb")
        brow1_sb = const.tile([1, hid], dt.bfloat16, tag="brow1_sb")
        brow2_sb = const.tile([1, hid], dt.bfloat16, tag="brow2_sb")
        iota_sb = const.tile([P, P], dt.bfloat16, tag="iota_sb")
        ident16_sb = const.tile([P, P], dt.bfloat16, tag="ident16_sb")
        idxlo_sb = const.tile([P, nblk * c_lo * 8], dt.int16, tag="idxlo_sb")
        idxhi_sb = const.tile([P, nblk * c_hi * 8], dt.int16, tag="idxhi_sb")
        dst_sb = const.tile([P, nblk * c_tot], dt.bfloat16, tag="dst_sb")
        dinv_sb = const.tile([P, nblk], dt.float32, tag="dinv_sb")
        dinvs_sb = const.tile([P, nblk], dt.float32, tag="dinvs_sb")
        dinvx_sb = const.tile([P, nblk], dt.float32, tag="dinvx_sb")
        binv_sb = const.tile([1, nshard], dt.bfloat16, tag="binv_sb")
        mask_sb = const.tile([P, nblk], dt.bfloat16, tag="mask_sb")

        zTa_sb = persist.tile([P, KH * na_sh], dt.bfloat16, tag="zTa_sb")
        zTb_sb = persist.tile([P, KH * nb_sh], dt.bfloat16, tag="zTb_sb")
        h1_sb = persist.tile([P, nblk * hid], dt.bfloat16, tag="h1_sb")
        h2_sb = persist.tile([P, nblk * hid], dt.bfloat16, tag="h2_sb")

        hshard_d = dram.tile([nshard, hid], dt.float8e4, tag="hshard_d")
        # Shared AG outputs must each have a single writer: one per layer.
        table1_d = dram.tile([npad, hid], dt.float8e4, tag="table1_d",
                             addr_space="Shared")
        table2_d = dram.tile([npad, hid], dt.float8e4, tag="table2_d",
                             addr_space="Shared")
        z1a_d = dram.tile([na_sh, hid], dt.bfloat16, tag="z1a_d")
        z1b_d = dram.tile([nb_sh, hid], dt.bfloat16, tag="z1b_d")

        # ---- constant loads ----
        nc.sync.dma_start(
            out=w1_sb[:, :].rearrange("p (k f) -> p k f", k=KIN),
            in_=w1[:, :].rearrange("(k p) f -> p k f", p=P))
        nc.sync.dma_start(
            out=w2_sb[:, :].rearrange("p (k f) -> p k f", k=KH),
            in_=w2[:, :].rearrange("(k p) f -> p k f", p=P))
        nc.sync.dma_start(out=brow1_sb[:, :], in_=brow1[:, :])
        nc.sync.dma_start(out=brow2_sb[:, :], in_=brow2[:, :])
        nc.sync.dma_start(out=iota_sb[:, :], in_=iota[:, :])
        nc.sync.dma_start(out=ident16_sb[:, :], in_=ident16[:, :])
        nc.sync.dma_start(out=idxlo_sb[:, :], in_=idxlo[:, :])
        nc.sync.dma_start(out=idxhi_sb[:, :], in_=idxhi[:, :])
        nc.sync.dma_start(out=dst_sb[:, :], in_=dstf[:, :])
        nc.sync.dma_start(out=dinv_sb[:, :], in_=dinv[:, :])
        nc.sync.dma_start(out=dinvs_sb[:, :], in_=dinvs[:, :])
        nc.sync.dma_start(out=dinvx_sb[:, :], in_=dinvx[:, :])
        nc.sync.dma_start(out=binv_sb[:, :], in_=binv[:, :])
        nc.sync.dma_start(out=mask_sb[:, :], in_=maskc[:, :])

        qcnt = [0]

        def next_q():
            q = qcnt[0] % NQ
            qcnt[0] += 1
            return q

        def dense_block(nb, lhs_src, w_sb, kc, scale_from_dinv, h_sb, h8c, lo_b):
            """h_sb/h8c[:, ...] = scale * (Z @ W) (bf16 / fp8 scaled 16x)."""
            ps = ps_mm.tile([P, hid], dt.float32, tag="mm")
            for k in range(kc):
                nc.tensor.matmul(
                    out=ps[:, :],
                    lhsT=lhs_src(nb, k),
                    rhs=w_sb[:, k * hid:(k + 1) * hid],
                    start=(k == 0), stop=(k == kc - 1))
            scale = dinv_sb[:, nb:nb + 1] if scale_from_dinv else 1.0
            scale16 = dinvx_sb[:, nb:nb + 1] if scale_from_dinv else FP8_SCALE
            nc.scalar.activation(
                h_sb[:, nb * hid:(nb + 1) * hid], ps[:, :],
                mybir.ActivationFunctionType.Copy, scale=scale)
            nc.scalar.activation(
                h8c[:, (nb - lo_b) * hid:(nb - lo_b + 1) * hid], ps[:, :],
                mybir.ActivationFunctionType.Copy, scale=scale16)

        def distribute(h8full, table_d):
            nc.sync.dma_start(
                out=hshard_d[:, :].rearrange("(nb p) f -> p nb f", p=P),
                in_=h8full[:, :].rearrange("p (nb f) -> p nb f", nb=nblk))
            nc.gpsimd.collective_compute(
                "AllGather", mybir.AluOpType.bypass,
                replica_groups=[list(range(N_CORES))],
                ins=[hshard_d[:, :].opt()],
                outs=[table_d[:, :].opt()])

        def aggregate(table_d, brow_sb, h_sb, z_consumer, post_block=None):
            for nb in range(nblk):
                mlo = msgp.tile([P, c_lo * hid], dt.float8e4, tag="mlo")
                mhi = msgp.tile([P, c_hi * hid], dt.float8e4, tag="mhi")
                nc.gpsimd.dma_gather(
                    out_ap=mlo[:, :].rearrange("p (c f) -> p c f", c=c_lo),
                    in_ap=table_d[0:half, :],
                    idxs_ap=idxlo_sb[:, nb * c_lo * 8:(nb + 1) * c_lo * 8],
                    num_idxs=c_lo * P,
                    num_idxs_reg=c_lo * P,
                    elem_size=hid, single_packet=False,
                    queue_num=next_q())
                nc.gpsimd.dma_gather(
                    out_ap=mhi[:, :].rearrange("p (c f) -> p c f", c=c_hi),
                    in_ap=table_d[half:npad, :],
                    idxs_ap=idxhi_sb[:, nb * c_hi * 8:(nb + 1) * c_hi * 8],
                    num_idxs=c_hi * P,
                    num_idxs_reg=c_hi * P,
                    elem_size=hid, single_packet=False,
                    queue_num=next_q())
                agg = ps_agg.tile([P, hid], dt.float32, tag="agg")
                st = stp.tile([P, c_tot * P], dt.float8e4, tag="st")
                nc.vector.tensor_tensor(
                    out=st[:, :].rearrange("p (c q) -> p c q", c=c_tot),
                    in0=dst_sb[:, nb * c_tot:(nb + 1) * c_tot]
                        .unsqueeze(2).to_broadcast([P, c_tot, P]),
                    in1=iota_sb[:, :].unsqueeze(1).to_broadcast([P, c_tot, P]),
                    op=mybir.AluOpType.is_equal)
                for c in range(c_lo):
                    nc.tensor.matmul(
                        out=agg[:, :], lhsT=st[:, c * P:(c + 1) * P],
                        rhs=mlo[:, c * hid:(c + 1) * hid],
                        start=(c == 0), stop=False)
                for c in range(c_hi):
                    nc.tensor.matmul(
                        out=agg[:, :], lhsT=st[:, (c_lo + c) * P:(c_lo + c + 1) * P],
                        rhs=mhi[:, c * hid:(c + 1) * hid],
                        start=False, stop=False)
                # bias as K=1 rank-1 update: 16*(1/dinv)_col x bias_row, so the
                # final dinv/16 scale reconstitutes a plain bias add.
                nc.tensor.matmul(
                    out=agg[:, :],
                    lhsT=binv_sb[0:1, nb * P:(nb + 1) * P],
                    rhs=brow_sb[0:1, :],
                    start=False, stop=False)
                nc.tensor.matmul(
                    out=agg[:, :], lhsT=ident16_sb[:, :],
                    rhs=h_sb[:, nb * hid:(nb + 1) * hid],
                    start=False, stop=True)
                z = zp.tile([P, hid], dt.bfloat16, tag="z")
                nc.scalar.activation(
                    z[:, :], agg[:, :], mybir.ActivationFunctionType.Relu,
                    scale=dinvs_sb[:, nb:nb + 1])
                z_consumer(nb, z)
                if post_block is not None:
                    post_block(nb)

        # ================= layer 1 dense + AG =================
        XB = 4  # xT blocks per lhs DMA
        xT_tiles = {}

        def xT_lhs(nb, k):
            nbg = nb - nb % XB
            key = (nbg, k)
            if key not in xT_tiles:
                w = min(XB, nblk - nbg)
                t = lhsp.tile([P, XB * P], dt.bfloat16, tag="xTt")
                nc.sync.dma_start(
                    out=t[:, :w * P],
                    in_=xT[k * P:(k + 1) * P, nbg * P:(nbg + w) * P])
                xT_tiles[key] = t
            return xT_tiles[key][:, (nb % XB) * P:(nb % XB + 1) * P]

        h8full1 = h8p.tile([P, nblk * hid], dt.float8e4, tag="h8full")
        for nb in range(nblk):
            dense_block(nb, xT_lhs, w1_sb, KIN, False, h1_sb, h8full1, 0)
        distribute(h8full1, table1_d)

        # ============ layer 1 aggregation (first-half mid chain hoisted) =====
        def z1_write(nb, z):
            if nb < ca_blk:
                nc.sync.dma_start(out=z1a_d[nb * P:(nb + 1) * P, :], in_=z[:, :])
            else:
                nb2 = nb - ca_blk
                nc.sync.dma_start(out=z1b_d[nb2 * P:(nb2 + 1) * P, :], in_=z[:, :])

        def zT_lhs(nb, k):
            if nb < ca_blk:
                return zTa_sb[:, k * na_sh + nb * P:k * na_sh + (nb + 1) * P]
            nb2 = nb - ca_blk
            return zTb_sb[:, k * nb_sh + nb2 * P:k * nb_sh + (nb2 + 1) * P]

        h8full2 = [None]
        dense2A_tr = min(ca_blk + 3, nblk - 1)

        def mid_phase(nb):
            # hoist the first half's transpose + layer-2 dense into the
            # layer-1 aggregation; no collective is issued mid-stream (it
            # would block the gather queue head while waiting on its inputs).
            if nb == ca_blk - 1:
                for h in range(KH):
                    nc.sync.dma_start(
                        out=zTa_sb[:, h * na_sh:(h + 1) * na_sh],
                        in_=z1a_d[:, h * P:(h + 1) * P],
                        transpose=True)
            if nb == dense2A_tr:
                h8full2[0] = h8p.tile([P, nblk * hid], dt.float8e4,
                                      tag="h8full", name="h8full2")
                for nb2 in range(ca_blk):
                    dense_block(nb2, zT_lhs, w2_sb, KH, True, h2_sb,
                                h8full2[0], 0)

        aggregate(table1_d, brow1_sb, h1_sb, z1_write, post_block=mid_phase)

        # remaining mid chain: second-half transpose, dense-2, one AG.
        for h in range(KH):
            nc.sync.dma_start(
                out=zTb_sb[:, h * nb_sh:(h + 1) * nb_sh],
                in_=z1b_d[:, h * P:(h + 1) * P],
                transpose=True)
        for nb2 in range(ca_blk, nblk):
            dense_block(nb2, zT_lhs, w2_sb, KH, True, h2_sb, h8full2[0], 0)
        distribute(h8full2[0], table2_d)

        # ================= layer 2 aggregation + readout =================
        cs = [ps_cs.tile([P, 1], dt.float32, tag=f"cs{h}", name=f"cs{h}")
              for h in range(KH)]

        def colsum(nb, z):
            for h in range(KH):
                nc.tensor.matmul(
                    out=cs[h][:, :], lhsT=z[:, h * P:(h + 1) * P],
                    rhs=mask_sb[:, nb:nb + 1],
                    start=(nb == 0), stop=(nb == nblk - 1))

        aggregate(table2_d, brow2_sb, h2_sb, colsum)

        out_sb = zp.tile([P, KH], dt.float32, tag="out_sb")
        for h in range(KH):
            nc.vector.tensor_copy(out=out_sb[:, h:h + 1], in_=cs[h][:, :])
        nc.sync.dma_start(out=out[:, :], in_=out_sb[:, :])

    nc.compile()
    return nc


# --------------------------------------------------------------------------- #
# Entry point
# --------------------------------------------------------------------------- #

_CACHE = {}


def _run(x, edge_index, W1, b1, W2, b2, trace=False):
    from concourse.bass_utils import run_bass_kernel_spmd

    in_maps, meta = _preprocess(x, edge_index, W1, b1, W2, b2)
    key = (meta["nblk"], meta["c_lo"], meta["c_hi"], meta["in_dim"], meta["hid"])
    if key not in _CACHE:
        _CACHE[key] = _build_nc(*key)
    nc = _CACHE[key]
    res = run_bass_kernel_spmd(
        nc, in_maps, core_ids=list(range(N_CORES)), trace=trace)
    parts = [r["partial"] for r in res.results]  # each [P, KH] f32
    colsum = np.sum(np.stack(parts), axis=0)     # [P, KH]
    g = colsum.T.reshape(-1)                     # [hid], g[h*P+p] = colsum[p, h]
    return g / float(meta["n"]), res


def kernel(x, edge_index, W1, b1, W2, b2, Wfc, bfc):
    x = np.asarray(x, dtype=np.float32)
    g, _ = _run(x, edge_index, np.asarray(W1, np.float32), np.asarray(b1, np.float32),
                np.asarray(W2, np.float32), np.asarray(b2, np.float32))
    logits = g.astype(np.float32) @ np.asarray(Wfc, np.float32) + np.asarray(bfc, np.float32)
    return (1.0 / (1.0 + np.exp(-logits))).astype(np.float32)
